# revision 27
# baseline (speedup 1.0000x reference)
"""Biaffine (trilinear + concat-linear) kernel for Trainium2, 8-core SPMD.

logits[b,x,y,o] = sum_ij in1[b,x,i] * w1[i,o,j] * in2[b,y,j]
               + termA[b,x,o] + termB[b,y,o] + bias[o]
  termA[b,x,o] = sum_i in1[b,x,i] * w2[i,o]
  termB[b,y,o] = sum_j in1[b,y,j] * w2[IN+j,o]   (both halves from input1!)
  bias[o]      = w2[2*IN,o]

Sharding: core c handles batch b=c//2, x-range [x0, x0+256), x0=256*(c%2).
w1/w2 replicated. Per core, two chained matmul phases over o-chunks of OC
(w1 is streamed through SBUF exactly once per core, batched OB o's per DMA,
host pre-casts it to bf16 to halve HBM traffic):
  phase 1: temp[j, o, x] = sum_i w1[i,o,j] * in1[x,i]
           (stationary = w1 128x128 tile, moving = in1^T [128, 256], fp32
           PSUM accumulation over 4 i-blocks, result stored bf16)
  phase 2: out[x, y] (per o) = sum_jblk temp-slice[j, x] @ in2T[j, y]
           + selector-matmul: lhsT[k,m] = identw[k,o] (free-broadcast AP)
             x rhs TBb[k,y]  ==> adds termB[y,o]+bias[o] to every x row
           then PSUM->SBUF copy fused with +termA[x,o] (tensor_scalar add)
temp is double-buffered so phase 1 of chunk N+1 overlaps phase 2 of chunk N.
Device output layout [x, o, y] so every output DMA line is >=14KB
contiguous; the host transposes to [x, y, o] while unsharding.

The selector matmul MUST use the bf16 identity (identw): with an fp32
zero-stride broadcast stationary the weight load takes a pathological slow
path and the whole main loop nearly doubles (measured ~0.96 ms -> ~0.6 ms
per core after switching it to bf16). Measured ~0.52-0.68 ms/core main loop
(repeat-delta wall-clock method, see bench.py); cost-model TimelineSim
estimates 0.47 ms.
"""

import numpy as np

B, S, IN, OUT = 4, 512, 512, 112
N_CORES = 8
P = 128


def split_sync_waits(nc, max_waits=1):
    """The walrus codegen in this toolchain rejects instructions carrying
    more than a few semaphore waits ("Too many sync wait commands").
    Hoist overflow waits onto NoOps inserted just before the instruction,
    on the same engine (semantically identical: the sequencer blocks on
    each wait in order)."""
    import concourse.mybir as mybir

    n_split = 0
    for f in nc.m.functions:
        for bb in f.blocks:
            new_insts = []
            for inst in bb.instructions:
                si = inst.sync_info
                if si is not None and si.on_wait and len(si.on_wait) > max_waits:
                    waits = list(si.on_wait)
                    overflow, keep = waits[:-max_waits], waits[-max_waits:]
                    for k in range(0, len(overflow), max_waits):
                        chunk = overflow[k:k + max_waits]
                        nop = mybir.InstNoOp(
                            name=f"{inst.name}_wsplit{k}",
                            opcode="NoOp",
                            engine=inst.engine,
                            sync_info=mybir.SyncInfo(on_wait=chunk, on_update=[]),
                        )
                        new_insts.append(nop)
                        n_split += 1
                    si.on_wait = keep
                new_insts.append(inst)
            bb.instructions[:] = new_insts
    return n_split


def build_nc(S_=S, IN_=IN, OUT_=OUT, XW=256, OC=14, OG=7, OB=7, w1_bf16=True,
             temp_bufs=2, split_waits=True, repeat=1, only_phase=0):
    """Build the per-core Bass module. All 8 cores run the same program on
    their own input slices (SPMD)."""
    import concourse.bass as bass
    import concourse.mybir as mybir
    import concourse.tile as tile
    from concourse.masks import make_identity

    f32 = mybir.dt.float32
    wdt = mybir.dt.bfloat16 if w1_bf16 else f32

    KI = IN_ // P          # number of 128-blocks of the i/j contraction dims
    YB = S_ // P           # y 128-blocks
    XB = XW // P           # x 128-blocks per core
    NCH = OUT_ // OC       # o-chunks
    assert OC % OG == 0 and OC % OB == 0

    nc = bass.Bass()
    in1x = nc.dram_tensor("in1x", [XW, IN_], f32, kind="ExternalInput")
    in1f = nc.dram_tensor("in1f", [S_, IN_], f32, kind="ExternalInput")
    in2f = nc.dram_tensor("in2f", [S_, IN_], f32, kind="ExternalInput")
    w1 = nc.dram_tensor("w1", [IN_, OUT_, IN_], wdt, kind="ExternalInput")
    w2 = nc.dram_tensor("w2", [2 * IN_ + 1, OUT_], f32, kind="ExternalInput")
    outp = nc.dram_tensor("outp", [XW, OUT_, S_], f32, kind="ExternalOutput")

    with tile.TileContext(nc) as tc:
        with tc.tile_pool(name="persist", bufs=1) as pers:
            # persistent SBUF tensors
            in1Tx = pers.tile([P, KI, XW], f32, name="in1Tx")   # in1x^T
            in1Tf = pers.tile([P, KI, S_], f32, name="in1Tf")   # in1f^T
            in2T = pers.tile([P, KI, S_], wdt, name="in2T")     # in2f^T
            wA = pers.tile([P, KI, OUT_], f32, name="wA")
            wB = pers.tile([P, KI, OUT_], f32, name="wB")
            biasc = pers.tile([OUT_, 1], f32, name="biasc")
            TBb = pers.tile([OUT_, S_], wdt, name="TBb")        # termB[y,o]+bias
            termA = pers.tile([P, XB, OUT_], f32, name="termA")
            ident = pers.tile([P, P], f32, name="ident")
            identw = pers.tile([P, P], wdt, name="identw")
            if w1_bf16:
                in1Tx_b = pers.tile([P, KI, XW], wdt, name="in1Tx_b")

            # ---------------- prep: transposes + affine terms ----------------
            with tc.tile_pool(name="prep", bufs=2) as prep, \
                 tc.tile_pool(name="prep_ps", bufs=2, space="PSUM") as prep_ps:
                make_identity(nc, ident)
                nc.vector.tensor_copy(identw, ident)

                nc.sync.dma_start(wA, w2[0:IN_, :].rearrange("(a p) o -> p a o", p=P))
                nc.sync.dma_start(wB, w2[IN_:2 * IN_, :].rearrange("(a p) o -> p a o", p=P))
                with nc.allow_non_contiguous_dma(reason="112B one-time bias load"):
                    nc.sync.dma_start(biasc, w2[2 * IN_:2 * IN_ + 1, :].rearrange("a o -> o a"))

                def transpose_into(dst, src_dram, rows):
                    # src_dram: [rows, IN_] fp32 -> dst [P, KI, rows] (= src^T)
                    st = prep.tile([P, rows // P, IN_], f32, name="stage", tag="stage")
                    nc.sync.dma_start(st, src_dram[:, :].rearrange("(a p) i -> p a i", p=P))
                    for a in range(rows // P):
                        for ib in range(KI):
                            pt = prep_ps.tile([P, P], f32, name="pt", tag="pt")
                            nc.tensor.transpose(pt, st[:, a, ib * P:(ib + 1) * P], ident)
                            nc.vector.tensor_copy(dst[:, ib, a * P:(a + 1) * P], pt)

                transpose_into(in1Tx, in1x, XW)
                transpose_into(in1Tf, in1f, S_)
                transpose_into(in2T, in2f, S_)  # cast to wdt in the copy
                if w1_bf16:
                    nc.vector.tensor_copy(in1Tx_b, in1Tx)

                # TBb[o, y] = sum_j wB[j,o] * in1f[y,j] + bias[o]
                psTB = prep_ps.tile([OUT_, S_], f32, name="psTB", tag="psTB")
                for jb in range(KI):
                    nc.tensor.matmul(psTB, wB[:, jb, :], in1Tf[:, jb, :],
                                     start=(jb == 0), stop=(jb == KI - 1))
                nc.vector.tensor_scalar_add(TBb, psTB, biasc)

                # termA[x, o] = sum_i in1x[x,i] * wA[i,o]
                for xb in range(XB):
                    psA = prep_ps.tile([P, OUT_], f32, name="psA", tag="psA")
                    for ib in range(KI):
                        nc.tensor.matmul(psA, in1Tx[:, ib, xb * P:(xb + 1) * P],
                                         wA[:, ib, :],
                                         start=(ib == 0), stop=(ib == KI - 1))
                    nc.vector.tensor_copy(termA[:, xb, :], psA)

            # ---------------- main: o-chunked two-phase pipeline ----------------
            with tc.tile_pool(name="w1p", bufs=8) as w1p, \
                 tc.tile_pool(name="tempp", bufs=temp_bufs) as tempp, \
                 tc.tile_pool(name="outsb", bufs=3) as outsb, \
                 tc.tile_pool(name="ps1", bufs=4, space="PSUM") as ps1p, \
                 tc.tile_pool(name="ps2", bufs=4, space="PSUM") as ps2p:
                rhs1 = in1Tx_b if w1_bf16 else in1Tx
                for oc in [c for _ in range(repeat) for c in range(NCH)]:
                    # phase 1: temp[j, ol, x] for this o-chunk
                    temp = tempp.tile([P, KI, OC, XW], wdt, name="temp", tag="temp")
                    for og in range(OC // OB) if only_phase in (0, 1) else []:
                        w1t = []
                        for ib in range(KI):
                            t = w1p.tile([P, OB, IN_], wdt, name="w1t", tag="w1t")
                            nc.sync.dma_start(
                                t, w1[ib * P:(ib + 1) * P,
                                      oc * OC + og * OB:oc * OC + (og + 1) * OB, :])
                            w1t.append(t)
                        for bl in range(OB):
                            ol = og * OB + bl
                            for jb in range(KI):
                                ps1 = ps1p.tile([P, XW], f32, name="ps1", tag="ps1")
                                for ib in range(KI):
                                    nc.tensor.matmul(
                                        ps1, w1t[ib][:, bl, jb * P:(jb + 1) * P],
                                        rhs1[:, ib, :],
                                        start=(ib == 0), stop=(ib == KI - 1))
                                nc.vector.tensor_copy(temp[:, jb, ol, :], ps1)
                    # phase 2: out[x, y] per o, + affine
                    for xb in range(XB) if only_phase in (0, 2) else []:
                        for g in range(OC // OG):
                            ot = outsb.tile([P, OG, S_], f32, name="ot", tag="ot")
                            for gl in range(OG):
                                ol = g * OG + gl
                                o = oc * OC + ol
                                ps2 = ps2p.tile([P, S_], f32, name="ps2", tag="ps2")
                                # selector matmul adds TBb[o, :] to every x row:
                                # lhsT[k, m] = ident[k, o] (free-broadcast), so
                                # out[m, n] += sum_k ident[k,o] * TBb[k,n] = TBb[o,n]
                                nc.tensor.matmul(
                                    ps2,
                                    identw[0:OUT_, o:o + 1].to_broadcast((OUT_, P)),
                                    TBb,
                                    start=True, stop=False)
                                for jb in range(KI):
                                    nc.tensor.matmul(
                                        ps2, temp[:, jb, ol, xb * P:(xb + 1) * P],
                                        in2T[:, jb, :],
                                        start=False, stop=(jb == KI - 1))
                                # PSUM->SBUF drain + termA add on the otherwise
                                # idle ACT engine (out = Copy(in*1 + bias)),
                                # keeping DVE free for the phase-1 copies
                                nc.scalar.activation(
                                    ot[:, gl, :], ps2,
                                    mybir.ActivationFunctionType.Identity,
                                    bias=termA[:, xb, o:o + 1])
                            nc.sync.dma_start(
                                outp[xb * P:(xb + 1) * P,
                                     oc * OC + g * OG:oc * OC + (g + 1) * OG, :],
                                ot)

    if split_waits:
        split_sync_waits(nc)
    return nc


_CACHE = {}


def _get_nc(**kw):
    key = tuple(sorted(kw.items()))
    if key not in _CACHE:
        _CACHE[key] = build_nc(**kw)
    return _CACHE[key]


W1_BF16 = True
TRACE = False
LAST_RESULT = None


def kernel(input1, input2, w1, w2, seq_len=None, **_ignored):
    global LAST_RESULT
    from concourse.bass_utils import run_bass_kernel_spmd
    import ml_dtypes

    input1 = np.asarray(input1, dtype=np.float32)
    input2 = np.asarray(input2, dtype=np.float32)
    w1 = np.asarray(w1, dtype=np.float32)
    w2 = np.asarray(w2, dtype=np.float32)

    nc = _get_nc(w1_bf16=W1_BF16)
    w1_dev = w1.astype(ml_dtypes.bfloat16) if W1_BF16 else w1

    XW = S // 2
    in_maps = []
    for c in range(N_CORES):
        b, xh = divmod(c, 2)
        x0 = xh * XW
        in_maps.append({
            "in1x": np.ascontiguousarray(input1[b, x0:x0 + XW, :]),
            "in1f": input1[b],
            "in2f": input2[b],
            "w1": w1_dev,
            "w2": w2,
        })
    res = run_bass_kernel_spmd(nc, in_maps, core_ids=list(range(N_CORES)),
                               trace=TRACE)
    LAST_RESULT = res

    full = np.empty((B, S, S, OUT), dtype=np.float32)
    for c in range(N_CORES):
        b, xh = divmod(c, 2)
        x0 = xh * XW
        # device layout [x, o, y] -> [x, y, o]
        full[b, x0:x0 + XW] = res.results[c]["outp"].transpose(0, 2, 1)
    return full


# revision 30
# speedup vs baseline: 4.6381x; 4.6381x over previous
"""Biaffine (trilinear + concat-linear) kernel for Trainium2, 8-core SPMD.

logits[b,x,y,o] = sum_ij in1[b,x,i] * w1[i,o,j] * in2[b,y,j]
               + termA[b,x,o] + termB[b,y,o] + bias[o]
  termA[b,x,o] = sum_i in1[b,x,i] * w2[i,o]
  termB[b,y,o] = sum_j in1[b,y,j] * w2[IN+j,o]   (both halves from input1!)
  bias[o]      = w2[2*IN,o]

Sharding: core c handles batch b=c//2, x-range [x0, x0+256), x0=256*(c%2).
w1/w2 replicated. Per core, two chained matmul phases over o-chunks of OC
(w1 is streamed through SBUF exactly once per core, batched OB o's per DMA,
host pre-casts it to bf16 to halve HBM traffic):
  phase 1: temp[j, o, x] = sum_i w1[i,o,j] * in1[x,i]
           (stationary = w1 128x128 tile, moving = in1^T [128, 256], fp32
           PSUM accumulation over 4 i-blocks, result stored bf16)
  phase 2: out[x, y] (per o) = sum_jblk temp-slice[j, x] @ in2T[j, y]
           + selector-matmul: lhsT[k,m] = identw[k,o] (free-broadcast AP)
             x rhs TBb[k,y]  ==> adds termB[y,o]+bias[o] to every x row
           then PSUM->SBUF copy fused with +termA[x,o] (tensor_scalar add)
temp is double-buffered so phase 1 of chunk N+1 overlaps phase 2 of chunk N.
Device output layout [x, o, y] so every output DMA line is >=14KB
contiguous; the host transposes to [x, y, o] while unsharding.

The selector matmul MUST use the bf16 identity (identw): with an fp32
zero-stride broadcast stationary the weight load takes a pathological slow
path and the whole main loop nearly doubles (measured ~0.96 ms -> ~0.6 ms
per core after switching it to bf16). Measured ~0.52-0.68 ms/core main loop
(repeat-delta wall-clock method, see bench.py); cost-model TimelineSim
estimates 0.47 ms.
"""

import numpy as np

B, S, IN, OUT = 4, 512, 512, 112
N_CORES = 8
P = 128


def split_sync_waits(nc, max_waits=1):
    """The walrus codegen in this toolchain rejects instructions carrying
    more than a few semaphore waits ("Too many sync wait commands").
    Hoist overflow waits onto NoOps inserted just before the instruction,
    on the same engine (semantically identical: the sequencer blocks on
    each wait in order)."""
    import concourse.mybir as mybir

    n_split = 0
    for f in nc.m.functions:
        for bb in f.blocks:
            new_insts = []
            for inst in bb.instructions:
                si = inst.sync_info
                if si is not None and si.on_wait and len(si.on_wait) > max_waits:
                    waits = list(si.on_wait)
                    overflow, keep = waits[:-max_waits], waits[-max_waits:]
                    for k in range(0, len(overflow), max_waits):
                        chunk = overflow[k:k + max_waits]
                        nop = mybir.InstNoOp(
                            name=f"{inst.name}_wsplit{k}",
                            opcode="NoOp",
                            engine=inst.engine,
                            sync_info=mybir.SyncInfo(on_wait=chunk, on_update=[]),
                        )
                        new_insts.append(nop)
                        n_split += 1
                    si.on_wait = keep
                new_insts.append(inst)
            bb.instructions[:] = new_insts
    return n_split


def build_nc(S_=S, IN_=IN, OUT_=OUT, XW=256, OC=14, OG=7, OB=7, w1_bf16=True,
             temp_bufs=2, split_waits=True, repeat=1, only_phase=0,
             act_drain=True):
    """Build the per-core Bass module. All 8 cores run the same program on
    their own input slices (SPMD)."""
    import concourse.bass as bass
    import concourse.mybir as mybir
    import concourse.tile as tile
    from concourse.masks import make_identity

    f32 = mybir.dt.float32
    wdt = mybir.dt.bfloat16 if w1_bf16 else f32

    KI = IN_ // P          # number of 128-blocks of the i/j contraction dims
    YB = S_ // P           # y 128-blocks
    XB = XW // P           # x 128-blocks per core
    NCH = OUT_ // OC       # o-chunks
    assert OC % OG == 0 and OC % OB == 0

    nc = bass.Bass()
    in1x = nc.dram_tensor("in1x", [XW, IN_], f32, kind="ExternalInput")
    in1f = nc.dram_tensor("in1f", [S_, IN_], f32, kind="ExternalInput")
    in2f = nc.dram_tensor("in2f", [S_, IN_], f32, kind="ExternalInput")
    w1 = nc.dram_tensor("w1", [IN_, OUT_, IN_], wdt, kind="ExternalInput")
    w2 = nc.dram_tensor("w2", [2 * IN_ + 1, OUT_], f32, kind="ExternalInput")
    outp = nc.dram_tensor("outp", [XW, OUT_, S_], f32, kind="ExternalOutput")

    with tile.TileContext(nc) as tc:
        with tc.tile_pool(name="persist", bufs=1) as pers:
            # persistent SBUF tensors
            in1Tx = pers.tile([P, KI, XW], f32, name="in1Tx")   # in1x^T
            in1Tf = pers.tile([P, KI, S_], f32, name="in1Tf")   # in1f^T
            in2T = pers.tile([P, KI, S_], wdt, name="in2T")     # in2f^T
            wA = pers.tile([P, KI, OUT_], f32, name="wA")
            wB = pers.tile([P, KI, OUT_], f32, name="wB")
            biasc = pers.tile([OUT_, 1], f32, name="biasc")
            TBb = pers.tile([OUT_, S_], wdt, name="TBb")        # termB[y,o]+bias
            termA = pers.tile([P, XB, OUT_], f32, name="termA")
            ident = pers.tile([P, P], f32, name="ident")
            identw = pers.tile([P, P], wdt, name="identw")
            if w1_bf16:
                in1Tx_b = pers.tile([P, KI, XW], wdt, name="in1Tx_b")

            # ---------------- prep: transposes + affine terms ----------------
            with tc.tile_pool(name="prep", bufs=2) as prep, \
                 tc.tile_pool(name="prep_ps", bufs=2, space="PSUM") as prep_ps:
                make_identity(nc, ident)
                nc.vector.tensor_copy(identw, ident)

                nc.sync.dma_start(wA, w2[0:IN_, :].rearrange("(a p) o -> p a o", p=P))
                nc.sync.dma_start(wB, w2[IN_:2 * IN_, :].rearrange("(a p) o -> p a o", p=P))
                with nc.allow_non_contiguous_dma(reason="112B one-time bias load"):
                    nc.sync.dma_start(biasc, w2[2 * IN_:2 * IN_ + 1, :].rearrange("a o -> o a"))

                def transpose_into(dst, src_dram, rows):
                    # src_dram: [rows, IN_] fp32 -> dst [P, KI, rows] (= src^T)
                    st = prep.tile([P, rows // P, IN_], f32, name="stage", tag="stage")
                    nc.sync.dma_start(st, src_dram[:, :].rearrange("(a p) i -> p a i", p=P))
                    for a in range(rows // P):
                        for ib in range(KI):
                            pt = prep_ps.tile([P, P], f32, name="pt", tag="pt")
                            nc.tensor.transpose(pt, st[:, a, ib * P:(ib + 1) * P], ident)
                            nc.vector.tensor_copy(dst[:, ib, a * P:(a + 1) * P], pt)

                transpose_into(in1Tx, in1x, XW)
                transpose_into(in1Tf, in1f, S_)
                transpose_into(in2T, in2f, S_)  # cast to wdt in the copy
                if w1_bf16:
                    nc.vector.tensor_copy(in1Tx_b, in1Tx)

                # TBb[o, y] = sum_j wB[j,o] * in1f[y,j] + bias[o]
                psTB = prep_ps.tile([OUT_, S_], f32, name="psTB", tag="psTB")
                for jb in range(KI):
                    nc.tensor.matmul(psTB, wB[:, jb, :], in1Tf[:, jb, :],
                                     start=(jb == 0), stop=(jb == KI - 1))
                nc.vector.tensor_scalar_add(TBb, psTB, biasc)

                # termA[x, o] = sum_i in1x[x,i] * wA[i,o]
                for xb in range(XB):
                    psA = prep_ps.tile([P, OUT_], f32, name="psA", tag="psA")
                    for ib in range(KI):
                        nc.tensor.matmul(psA, in1Tx[:, ib, xb * P:(xb + 1) * P],
                                         wA[:, ib, :],
                                         start=(ib == 0), stop=(ib == KI - 1))
                    nc.vector.tensor_copy(termA[:, xb, :], psA)

            # ---------------- main: o-chunked two-phase pipeline ----------------
            with tc.tile_pool(name="w1p", bufs=8) as w1p, \
                 tc.tile_pool(name="tempp", bufs=temp_bufs) as tempp, \
                 tc.tile_pool(name="outsb", bufs=3) as outsb, \
                 tc.tile_pool(name="ps1", bufs=4, space="PSUM") as ps1p, \
                 tc.tile_pool(name="ps2", bufs=4, space="PSUM") as ps2p:
                rhs1 = in1Tx_b if w1_bf16 else in1Tx
                for oc in [c for _ in range(repeat) for c in range(NCH)]:
                    # phase 1: temp[j, ol, x] for this o-chunk
                    temp = tempp.tile([P, KI, OC, XW], wdt, name="temp", tag="temp")
                    for og in range(OC // OB) if only_phase in (0, 1) else []:
                        w1t = []
                        for ib in range(KI):
                            t = w1p.tile([P, OB, IN_], wdt, name="w1t", tag="w1t")
                            nc.sync.dma_start(
                                t, w1[ib * P:(ib + 1) * P,
                                      oc * OC + og * OB:oc * OC + (og + 1) * OB, :])
                            w1t.append(t)
                        for bl in range(OB):
                            ol = og * OB + bl
                            for jb in range(KI):
                                ps1 = ps1p.tile([P, XW], f32, name="ps1", tag="ps1")
                                for ib in range(KI):
                                    nc.tensor.matmul(
                                        ps1, w1t[ib][:, bl, jb * P:(jb + 1) * P],
                                        rhs1[:, ib, :],
                                        start=(ib == 0), stop=(ib == KI - 1))
                                nc.vector.tensor_copy(temp[:, jb, ol, :], ps1)
                    # phase 2: out[x, y] per o, + affine
                    for xb in range(XB) if only_phase in (0, 2) else []:
                        for g in range(OC // OG):
                            ot = outsb.tile([P, OG, S_], f32, name="ot", tag="ot")
                            for gl in range(OG):
                                ol = g * OG + gl
                                o = oc * OC + ol
                                ps2 = ps2p.tile([P, S_], f32, name="ps2", tag="ps2")
                                # selector matmul adds TBb[o, :] to every x row:
                                # lhsT[k, m] = ident[k, o] (free-broadcast), so
                                # out[m, n] += sum_k ident[k,o] * TBb[k,n] = TBb[o,n]
                                nc.tensor.matmul(
                                    ps2,
                                    identw[0:OUT_, o:o + 1].to_broadcast((OUT_, P)),
                                    TBb,
                                    start=True, stop=False)
                                for jb in range(KI):
                                    nc.tensor.matmul(
                                        ps2, temp[:, jb, ol, xb * P:(xb + 1) * P],
                                        in2T[:, jb, :],
                                        start=False, stop=(jb == KI - 1))
                                if act_drain:
                                    # PSUM drain + termA add on the idle ACT
                                    # engine: out = Identity(in*1 + bias)
                                    nc.scalar.activation(
                                        ot[:, gl, :], ps2,
                                        mybir.ActivationFunctionType.Identity,
                                        bias=termA[:, xb, o:o + 1])
                                else:
                                    nc.vector.tensor_scalar_add(
                                        ot[:, gl, :], ps2,
                                        termA[:, xb, o:o + 1])
                            nc.sync.dma_start(
                                outp[xb * P:(xb + 1) * P,
                                     oc * OC + g * OG:oc * OC + (g + 1) * OG, :],
                                ot)

    if split_waits:
        split_sync_waits(nc)
    return nc


_CACHE = {}


def _get_nc(**kw):
    key = tuple(sorted(kw.items()))
    if key not in _CACHE:
        _CACHE[key] = build_nc(**kw)
    return _CACHE[key]


W1_BF16 = True
TRACE = False
LAST_RESULT = None


def kernel(input1, input2, w1, w2, seq_len=None, **_ignored):
    global LAST_RESULT
    from concourse.bass_utils import run_bass_kernel_spmd
    import ml_dtypes

    input1 = np.asarray(input1, dtype=np.float32)
    input2 = np.asarray(input2, dtype=np.float32)
    w1 = np.asarray(w1, dtype=np.float32)
    w2 = np.asarray(w2, dtype=np.float32)

    nc = _get_nc(w1_bf16=W1_BF16)
    w1_dev = w1.astype(ml_dtypes.bfloat16) if W1_BF16 else w1

    XW = S // 2
    in_maps = []
    for c in range(N_CORES):
        b, xh = divmod(c, 2)
        x0 = xh * XW
        in_maps.append({
            "in1x": np.ascontiguousarray(input1[b, x0:x0 + XW, :]),
            "in1f": input1[b],
            "in2f": input2[b],
            "w1": w1_dev,
            "w2": w2,
        })
    res = run_bass_kernel_spmd(nc, in_maps, core_ids=list(range(N_CORES)),
                               trace=TRACE)
    LAST_RESULT = res

    full = np.empty((B, S, S, OUT), dtype=np.float32)
    for c in range(N_CORES):
        b, xh = divmod(c, 2)
        x0 = xh * XW
        # device layout [x, o, y] -> [x, y, o]
        full[b, x0:x0 + XW] = res.results[c]["outp"].transpose(0, 2, 1)
    return full
